# revision 35
# baseline (speedup 1.0000x reference)
"""FeatureProcessingBlock Trainium kernel.

out = sum_t einsum('bcphqw,twW,thH,tcC->bCpHqW', x.reshape(B,C,P,64,Q,64), Ws, Hs, Cs)

Sharding: 8 cores = (B=4) x (H-halves=2); each core gets x[b, :, ph*256:(ph+1)*256, :]
i.e. a [C=48, 256, 512] slab = 4 p-blocks x 8 q-blocks of 64x64 windows.

Per-core pipeline (single pass over HBM, f32r matmuls):
  h-stage:  tiles [h | (c,w)] -> Y = Hs_t^T X        (K=64 matmul, H' on psum partitions)
  swap1:    DVE stream-transpose (32x32 blocks) PSUM->SBUF: w-sub on partitions
  w-stage:  K=32 matmuls (row-groups by H'-half), accumulate over w-halves
  swap2:    stream-transpose PSUM->SBUF: c on partitions
  c-stage:  K=32 matmuls (row-groups by W'-half), accumulate over t and c-halves
            (c 48..63 are duplicated junk lanes killed by zero rows in the Cs tile)
  exit:     copy psum->SBUF, DMA out
"""

import numpy as np

B, C, H, W = 4, 48, 512, 512
T, WS = 3, 64
NCORES = 8
PH = H // 2  # rows per core

LAST_EXEC_NS = None

_CACHE = {}


def _build(np_blocks, nq_blocks):
    """Build the bass module for a shard of [C, np_blocks*64, nq_blocks*64]."""
    import concourse.bacc as bacc
    import concourse.mybir as mybir
    from concourse.bass import MemorySpace
    from concourse.tile import TileContext

    F32 = mybir.dt.float32
    F32R = mybir.dt.float32r

    HS_ROWS = np_blocks * 64
    WS_COLS = nq_blocks * 64

    nc = bacc.Bacc("TRN2", target_bir_lowering=False, debug=False, num_devices=NCORES)
    x = nc.dram_tensor("x", [C, HS_ROWS, WS_COLS], F32R, kind="ExternalInput")
    hs_d = nc.dram_tensor("hs", [T, 64, 64], F32R, kind="ExternalInput")
    ws_d = nc.dram_tensor("ws", [T, 64, 64], F32R, kind="ExternalInput")
    cs_d = nc.dram_tensor("cs", [T, C, C], F32R, kind="ExternalInput")
    out = nc.dram_tensor("out", [C, HS_ROWS, WS_COLS], F32, kind="ExternalOutput")

    with TileContext(nc) as tc:
        with (
            tc.tile_pool(name="consts", bufs=1) as consts,
            tc.tile_pool(name="xin", bufs=4) as xin,
            tc.tile_pool(name="vbuf", bufs=3) as vbuf,
            tc.tile_pool(name="ubuf", bufs=3) as ubuf,
            tc.tile_pool(name="obuf", bufs=3) as obuf,
            tc.tile_pool(name="ypsum", bufs=2, space=MemorySpace.PSUM) as ypsum,
            tc.tile_pool(name="zpsum", bufs=4, space=MemorySpace.PSUM) as zpsum,
            tc.tile_pool(name="opsum", bufs=1, space=MemorySpace.PSUM) as opsum,
        ):
            # ---- constant factor tiles ----
            # Hs: [h | (t, H')]
            hs_sb = consts.tile([64, T, 64], F32R)
            nc.sync.dma_start(out=hs_sb, in_=hs_d[:, :, :].rearrange("t h H -> h t H"))
            # Ws replicated on all 4 partition groups: [32g + wsub | (t, wb, W')]
            ws_sb = consts.tile([128, T, 2, 64], F32R)
            for g in range(4):
                nc.sync.dma_start(
                    out=ws_sb[32 * g : 32 * g + 32],
                    in_=ws_d[:, :, :].rearrange("t (wb u) W -> u t wb W", wb=2),
                )
            # Cs stream-A rows c0..31, replicated on groups 0-1
            csa_sb = consts.tile([64, T, C], F32R)
            for g in range(2):
                nc.sync.dma_start(
                    out=csa_sb[32 * g : 32 * g + 32],
                    in_=cs_d[:, 0:32, :].rearrange("t c C -> c t C"),
                )
            # Cs stream-B: rows 0-15 = Cs[t, 32:48], rows 16-31 zero
            csb_sb = consts.tile([64, T, C], F32R)
            nc.any.memzero(csb_sb)
            for g in range(2):
                nc.sync.dma_start(
                    out=csb_sb[32 * g : 32 * g + 16],
                    in_=cs_d[:, 32:48, :].rearrange("t c C -> c t C"),
                )

            NCC = C // 16  # 3 x-chunks of 16 channels
            for p in range(np_blocks):
                for q in range(nq_blocks):
                    # ---- load window stack: 3 chunks [h | (c16, w64)] ----
                    xch = []
                    for cc in range(NCC):
                        xt = xin.tile([64, 16, 64], F32R, tag="x")
                        nc.sync.dma_start(
                            out=xt,
                            in_=x[
                                16 * cc : 16 * cc + 16,
                                64 * p : 64 * p + 64,
                                64 * q : 64 * q + 64,
                            ].rearrange("c h w -> h c w"),
                        )
                        xch.append(xt)

                    # ---- h-stage + swap1 ----
                    # V[t][part = 32*Hh + wsub | (c64, wb2, hsub32)]; c 48..63 dup junk
                    vt_tiles = []
                    for t in range(T):
                        vt = vbuf.tile([64, 64, 2, 32], F32, tag="v")
                        for yc in range(2 * NCC):
                            cc, sub = yc // 2, yc % 2
                            y = ypsum.tile([64, 8, 64], F32, tag="y")
                            nc.tensor.matmul(
                                y,
                                hs_sb[:, t, :],
                                xch[cc][:, 8 * sub : 8 * sub + 8, :],
                                start=True,
                                stop=True,
                            )
                            # swap1: blocks (c8, wb2); partitions get wsub
                            nc.vector.transpose(
                                out=vt[:, 8 * yc : 8 * yc + 8, :, :], in_=y
                            )
                            if yc >= 4:  # duplicate c 32..47 into pad slots 48..63
                                nc.vector.transpose(
                                    out=vt[:, 8 * yc + 16 : 8 * yc + 24, :, :], in_=y
                                )
                        v2 = vbuf.tile([64, 64, 2, 32], F32R, tag="v2")
                        nc.any.tensor_copy(out=v2, in_=vt)
                        vt_tiles.append(v2)

                    # ---- per H'-half: w-stage, swap2, c-stage ----
                    for hh in range(2):
                        ua = {}
                        ub = {}
                        for t in range(T):
                            # four 1-bank psum tiles: (stream, hq)
                            za = [
                                zpsum.tile([64, 16, 32], F32, tag="z", name=f"za{i}")
                                for i in range(2)
                            ]
                            zb = [
                                zpsum.tile([64, 16, 32], F32, tag="z", name=f"zb{i}")
                                for i in range(2)
                            ]
                            for wb in range(2):
                                lhs = ws_sb[32 * hh : 32 * hh + 32, t, wb, :]
                                rhs_a = vt_tiles[t][
                                    32 * hh : 32 * hh + 32, 0:32, wb, :
                                ].rearrange("p c h -> p h c")
                                rhs_b = vt_tiles[t][
                                    32 * hh : 32 * hh + 32, 32:64, wb, :
                                ].rearrange("p c h -> p h c")
                                for hq in range(2):
                                    nc.tensor.matmul(
                                        za[hq],
                                        lhs,
                                        rhs_a[:, 16 * hq : 16 * hq + 16, :],
                                        start=(wb == 0),
                                        stop=(wb == 1),
                                        tile_position=(32 * hh, 0),
                                    )
                                    nc.tensor.matmul(
                                        zb[hq],
                                        lhs,
                                        rhs_b[:, 16 * hq : 16 * hq + 16, :],
                                        start=(wb == 0),
                                        stop=(wb == 1),
                                        tile_position=(32 * hh, 0),
                                    )
                            # swap2: partitions get c-sub32
                            uat = ubuf.tile([64, 32, 32], F32, tag="ua")
                            ubt = ubuf.tile([64, 32, 32], F32, tag="ub")
                            for hq in range(2):
                                nc.vector.transpose(
                                    out=uat[:, 16 * hq : 16 * hq + 16, :], in_=za[hq]
                                )
                                nc.vector.transpose(
                                    out=ubt[:, 16 * hq : 16 * hq + 16, :], in_=zb[hq]
                                )
                            uat2 = ubuf.tile([64, 32, 32], F32R, tag="ua2")
                            nc.any.tensor_copy(out=uat2, in_=uat)
                            ubt2 = ubuf.tile([64, 32, 32], F32R, tag="ub2")
                            nc.any.tensor_copy(out=ubt2, in_=ubt)
                            ua[t] = uat2
                            ub[t] = ubt2

                        for hq in range(2):
                            # free = (vb, hsub, wsub): each vb-half is one psum bank
                            o_ps = opsum.tile([C, 2, 16, 32], F32, tag="o")
                            chain = []
                            for t in range(T):
                                chain.append((csa_sb, ua[t], t))
                                chain.append((csb_sb, ub[t], t))
                            for vb in range(2):
                                for ci, (cmat, u, t) in enumerate(chain):
                                    nc.tensor.matmul(
                                        o_ps[:, vb, :, :],
                                        cmat[32 * vb : 32 * vb + 32, t, :],
                                        u[
                                            32 * vb : 32 * vb + 32,
                                            16 * hq : 16 * hq + 16,
                                            :,
                                        ],
                                        start=(ci == 0),
                                        stop=(ci == len(chain) - 1),
                                        tile_position=(32 * vb, 0),
                                    )
                            o_sb = obuf.tile([C, 16, 2, 32], F32, tag="os")
                            # reorder (vb, hsub, wsub) -> (hsub, vb, wsub) during the exit copy
                            nc.any.tensor_copy(
                                out=o_sb.rearrange("p h v w -> p v h w"), in_=o_ps
                            )
                            r0 = 64 * p + 32 * hh + 16 * hq
                            nc.sync.dma_start(
                                out=out[:, r0 : r0 + 16, 64 * q : 64 * q + 64],
                                in_=o_sb,
                            )

    nc.compile()
    return nc


def _get_nc(np_blocks, nq_blocks):
    key = (np_blocks, nq_blocks)
    if key not in _CACHE:
        _CACHE[key] = _build(np_blocks, nq_blocks)
    return _CACHE[key]


def kernel(x, Ws, Hs, Cs, window_size):
    global LAST_EXEC_NS
    from concourse.bass_utils import run_bass_kernel_spmd

    x = np.asarray(x, dtype=np.float32)
    Ws = np.asarray(Ws, dtype=np.float32)
    Hs = np.asarray(Hs, dtype=np.float32)
    Cs = np.asarray(Cs, dtype=np.float32)
    assert int(window_size) == WS
    assert x.shape == (B, C, H, W)

    nc = _get_nc(4, 8)
    in_maps = []
    for core in range(NCORES):
        b, ph = core // 2, core % 2
        shard = np.ascontiguousarray(x[b, :, ph * PH : (ph + 1) * PH, :])
        in_maps.append({"x": shard, "hs": Hs, "ws": Ws, "cs": Cs})

    res = run_bass_kernel_spmd(nc, in_maps, core_ids=list(range(NCORES)))
    LAST_EXEC_NS = res.exec_time_ns

    full = np.empty((B, C, H, W), dtype=np.float32)
    for core in range(NCORES):
        b, ph = core // 2, core % 2
        full[b, :, ph * PH : (ph + 1) * PH, :] = res.results[core]["out"]
    return full

